# revision 12
# baseline (speedup 1.0000x reference)
"""GAT layer (4096 nodes, 8 heads, 64 feat/head) on 8 Trainium2 NeuronCores.

Sharding: node dim i (rows of x/adj/output) split 8 ways; W/a replicated;
each core computes attention+aggregation for its 512-row block against all
4096 j-nodes (Wh computed redundantly per core from the full x).

Math (per core, per head h):
  s_i = e_src[i,h], d_j = e_dst[j,h]
  exp(lrelu(s+d)) = exp(0.2 s_i) * max(q_j, E8_i * expd_j)
  with q_j = exp(0.2 d_j), E8_i = exp(0.8 s_i), expd_j = exp(d_j);
  the factor exp(0.2 s_i) cancels in the softmax ratio, so
  g[j,i] = adj[i,j] * max(q_j, E8_i * expd_j) is the unnormalized attention,
  out[i,d] = relu( (sum_j g[j,i] Wh[j,d]) / (sum_j g[j,i]) ),
  both sums from ONE matmul per (j-chunk, head) with rhs=g, lhsT=[Wh_h|ones].

The [128j x 512i x 4head] g tiles are produced by THREE engine routes,
balanced so DVE / Act / Pool all finish together (the adjacency select is
the whole cost: 2 dense elementwise passes if done naively on DVE):
  D-route (DVE): dual-op tensor_scalar m = max(E8*expd, q) per head (4x
      mode, ~194ns) + one 4-head-batched mask mult g = m*adj (2x, ~1.1us).
  G-route (Pool): the ~1% adjacency density makes g tiles sparse, so
      gpsimd local_scatter builds them directly: it ZEROES the dest and
      writes only the ~5 edges/row (pad idx=-1 is ignored). The host
      pre-gathers the edge E8 values (pure indexing, no math); two tiny
      [128, 4K] DVE tensor_tensor ops (~110ns each) apply expd and the
      max-with-q on the edge values; two 2-head scatters (~1.5us each)
      materialize the masked tiles. No adj DMA for all-G chunks.
  A-route (Act): m = max(E8*expd, q) as Relu(E8*expd - q) then
      Relu(r + q) (scale/bias are per-partition APs), mask mult on DVE.
Per-route per-pass engine cost: D 1903ns DVE; G ~3.1us Pool + 220ns DVE;
A ~4.9us Act + 1.1us DVE.  The Pool engine runs ONLY local_scatter ops
mid-kernel (single ucode library -> one reload, pulled to t~1us by a dummy
scatter; identity/ones memsets run before it in the standard library).

Schedule: Wh staging (x@W + ACT copies into bf16 [Wh_h|1] lhsT blocks) is
software-pipelined into the sweep-1 passes (lookahead 4); pre-work of each
route is emitted ts_ahead/g_ahead passes early; finalize (PE transpose,
DVE reciprocal, ACT/DVE relu-scale) is interleaved into sweep 2; 4
identity-transpose warmups hold the PE p-state through the DMA window.
bf16 g/Wh keep ~3e-3 max rel error (gate 2e-2); GAT_EXACT=1 switches to
f32/f32r and routes every pass through the D path.
"""

import sys

sys.path.insert(0, "/opt/trn_rl_repo")

import ml_dtypes
import numpy as np

import concourse.bass as bass
import concourse.mybir as mybir
import concourse.tile as tile
from concourse import bacc
from concourse.bass_utils import run_bass_kernel_spmd
from concourse.masks import make_identity

N_NODES = 4096
IN_FEAT = 256
OUT_FEAT = 64
N_HEADS = 8
N_CORES = 8
ROWS = N_NODES // N_CORES          # 512 i-rows per core
F = N_HEADS * OUT_FEAT             # 512
JCH = N_NODES // 128               # 32 j-chunks
KCH = IN_FEAT // 128               # 2 k-chunks
E = OUT_FEAT + 1                   # 65: per-head lhsT cols (Wh | ones)
KPAD = 28                          # padded edge slots per (j, core-block)

f32 = mybir.dt.float32
f32r = mybir.dt.float32r
bf16 = mybir.dt.bfloat16
i16 = mybir.dt.int16
AF = mybir.ActivationFunctionType
OP = mybir.AluOpType


def route_plan(n_gjc=12, n_a=9):
    """Static per-pass route assignment. Passes are (head-group, jc) for
    hg in {0,1}, jc in 0..31, with the final pass split into head pairs.
    G is assigned by WHOLE jc (both head groups) so those chunks never
    need their adjacency tile at all."""
    if n_gjc > 0:
        g_jcs = sorted({int(round(1 + i * 29.0 / max(1, n_gjc - 1)))
                        for i in range(n_gjc)})
    else:
        g_jcs = []
    routes = {}
    for hg in range(2):
        for jc in range(JCH):
            routes[(hg, jc)] = "G" if jc in g_jcs else "D"
    # A-passes: late in each sweep, on non-G chunks, avoiding the tail
    cand = [(hg, jc) for hg in range(2) for jc in range(16, 30)
            if jc not in g_jcs]
    cand = cand[:: max(1, len(cand) // n_a)][:n_a] if n_a else []
    for key in cand:
        routes[key] = "A"
    # tail passes stay D (final chunk feeds the exit path)
    for hg in range(2):
        routes[(hg, JCH - 1)] = "D"
    return routes, g_jcs


def build_nc(exact=False, kpad=KPAD, n_gjc=12, n_a=9, lookahead=4,
             ts_ahead=2, g_ahead=4, a_ahead=3, n_warm=4, sa_dve=True,
             fin0=4, fin_step=5, dma0=26, repeat=1):
    if exact:
        n_gjc, n_a = 0, 0
    routes, g_jcs = route_plan(n_gjc, n_a)
    K2 = 2 * kpad                       # idx slots per 2-head scatter
    mm_dt = f32r if exact else bf16
    wh_dt = f32 if exact else f32r
    nc = bacc.Bacc("TRN2", target_bir_lowering=False, debug=False,
                   num_devices=N_CORES)
    xT = nc.dram_tensor("xT", [IN_FEAT, N_NODES], wh_dt, kind="ExternalInput").ap()
    esrcD = nc.dram_tensor("esrc", [8, ROWS], wh_dt, kind="ExternalInput").ap()
    evqdD = nc.dram_tensor("evqd", [N_NODES // 8, 128], f32,
                           kind="ExternalInput").ap()
    Wd = nc.dram_tensor("W", [IN_FEAT, F], wh_dt, kind="ExternalInput").ap()
    selD = nc.dram_tensor("sel", [8, N_HEADS * 128], wh_dt,
                          kind="ExternalInput").ap()
    adjT = nc.dram_tensor("adjT", [N_NODES, ROWS], bf16, kind="ExternalInput").ap()
    out = nc.dram_tensor("out", [ROWS, F], f32, kind="ExternalOutput").ap()
    g_passes = [(hg, jc) for hg in range(2) for jc in range(JCH)
                if routes[(hg, jc)] == "G"]
    if g_passes:
        # edge E8 values (host-gathered), per G-pass [128, 4*kpad] bf16
        egD = {k: nc.dram_tensor(f"eg{k[0]}_{k[1]}", [128, 4 * kpad], bf16,
                                 kind="ExternalInput").ap() for k in g_passes}
        idxD = {jc: nc.dram_tensor(f"idx{jc}", [128, K2], i16,
                                   kind="ExternalInput").ap()
                for jc in sorted(g_jcs)}
        # expd/q patterns [128, jc*8+h] bf16 (tile layout of exp(e_dst))
        epatD = nc.dram_tensor("epat", [128, JCH * 8], bf16,
                               kind="ExternalInput").ap()
        qpatD = nc.dram_tensor("qpat", [128, JCH * 8], bf16,
                               kind="ExternalInput").ap()
    if n_a > 0:
        # -q in f32 tile layout, bias AP for the A-route relu chain
        nqpatD = nc.dram_tensor("nqpat", [128, JCH * 8], f32,
                                kind="ExternalInput").ap()

    with tile.TileContext(nc) as tc:
      for rep in range(repeat):
        with tc.tile_pool(name=f"persist{rep}", bufs=1) as per, \
             tc.tile_pool(name=f"ph1sb{rep}", bufs=1) as ph1, \
             tc.tile_pool(name=f"work{rep}", bufs=1) as work, \
             tc.tile_pool(name=f"fin{rep}", bufs=1) as fin:
            # persistent SBUF (unique tags -> dedicated slots)
            wh1 = [per.tile([128, N_HEADS * E], mm_dt, tag=f"wh1_{jc}", name=f"wh1_{jc}")
                   for jc in range(JCH)]
            adjs = {jc: per.tile([128, ROWS], bf16, tag=f"adj_{jc}",
                                 name=f"adj_{jc}")
                    for jc in range(JCH) if jc not in g_jcs}
            evqd_g = [per.tile([128, 128], f32, tag=f"evqd_g{g}",
                               name=f"evqd_g{g}") for g in range(4)]
            e8 = [per.tile([128, ROWS], mm_dt, tag=f"e8_{h}", name=f"e8_{h}")
                  for h in range(N_HEADS)]
            outsb = [per.tile([128, F], f32, tag=f"out_{k}", name=f"out_{k}")
                     for k in range(4)]
            espall = per.tile([8, ROWS], wh_dt, tag="espall", name="espall")
            # sel[k, h, p] = (k == h): one [8,128] slice per head is a
            # row-selector lhsT -- matmul(sel_h, espall) both selects row h
            # and broadcasts it across 128 partitions in one K=8 matmul.
            sel = per.tile([8, N_HEADS * 128], wh_dt, tag="sel", name="sel")
            ones128 = per.tile([128, 8], f32, tag="ones128", name="ones128")
            ident = per.tile([128, 128], f32, tag="ident", name="ident")
            if g_passes:
                egs = {k: per.tile([128, 4 * kpad], bf16,
                                   tag=f"eg_{k[0]}_{k[1]}",
                                   name=f"eg_{k[0]}_{k[1]}") for k in g_passes}
                idxs = {jc: per.tile([128, K2], i16, tag=f"idx_{jc}",
                                     name=f"idx_{jc}")
                        for jc in sorted(g_jcs)}
                epat = per.tile([128, JCH * 8], bf16, tag="epat", name="epat")
                qpat = per.tile([128, JCH * 8], bf16, tag="qpat", name="qpat")
                dum = per.tile([16, 2], i16, tag="dum", name="dum")
                dumo = per.tile([16, 2], bf16, tag="dumo", name="dumo")
            if n_a > 0:
                nqpat = per.tile([128, JCH * 8], f32, tag="nqpat",
                                 name="nqpat")

            xt_sb = [ph1.tile([128, N_NODES], wh_dt, tag=f"xt{k}", name=f"xt{k}")
                     for k in range(KCH)]
            w_sb = [ph1.tile([128, F], wh_dt, tag=f"w{k}", name=f"w{k}")
                    for k in range(KCH)]

            nc.gpsimd.memset(ones128[:], 1.0)
            make_identity(nc, ident[:])
            if g_passes:
                # dummy scatter pulls the local_scatter library reload (and
                # the one-time Q7 bringup) to t~1us, before real work needs
                # the Pool engine
                nc.gpsimd.memset(dum[:], -1)
                nc.gpsimd.local_scatter(dumo[:], dum[:], dum[:], channels=16,
                                        num_elems=2, num_idxs=2)

            # --- input DMAs, all on the cheap sync queue. Gating chain
            # first (host-computed exp(0.8 e_src) + selector feed the E8
            # broadcast; per-j scalar/pattern tables land next), then first
            # adj/idx/eg/xT/W, then the rest interleaved.
            nc.sync.dma_start(espall[:], esrcD)
            nc.sync.dma_start(sel[:], selD)
            nc.sync.dma_start(evqd_g[0][:], evqdD[0:128, :])
            if g_passes:
                nc.sync.dma_start(epat[:], epatD)
                nc.sync.dma_start(qpat[:], qpatD)
                for jc in sorted(g_jcs)[:2]:
                    nc.sync.dma_start(idxs[jc][:], idxD[jc])
            if n_a > 0:
                nc.sync.dma_start(nqpat[:], nqpatD)
            for jc in range(2):
                if jc in adjs:
                    nc.sync.dma_start(adjs[jc][:],
                                      adjT[jc * 128:(jc + 1) * 128, :])
            for k in range(KCH):
                sl = slice(k * 128, (k + 1) * 128)
                nc.sync.dma_start(xt_sb[k][:][:, 0:512], xT[sl, 0:512])
            for k in range(KCH):
                sl = slice(k * 128, (k + 1) * 128)
                nc.sync.dma_start(w_sb[k][:], Wd[sl, :])
            for jc in range(2, 4):
                if jc in adjs:
                    nc.sync.dma_start(adjs[jc][:],
                                      adjT[jc * 128:(jc + 1) * 128, :])
            if g_passes:
                for jc in sorted(g_jcs)[2:]:
                    nc.sync.dma_start(idxs[jc][:], idxD[jc])
                for k in [k for k in g_passes if k[0] == 0]:
                    nc.sync.dma_start(egs[k][:], egD[k])
            for g in range(1, 4):
                nc.sync.dma_start(evqd_g[g][:],
                                  evqdD[g * 128:(g + 1) * 128, :])
            for c in range(1, 8):
                csl = slice(c * 512, (c + 1) * 512)
                for k in range(KCH):
                    sl = slice(k * 128, (k + 1) * 128)
                    nc.sync.dma_start(xt_sb[k][:][:, csl], xT[sl, csl])
                for jc in range(4 * c, 4 * c + 4):
                    if jc in adjs:
                        nc.sync.dma_start(adjs[jc][:],
                                          adjT[jc * 128:(jc + 1) * 128, :])
            if g_passes:
                for k in [k for k in g_passes if k[0] == 1]:
                    nc.sync.dma_start(egs[k][:], egD[k])

            # --- main: two sweeps over j-chunks (head groups 0-3, 4-7),
            # Wh/e-vector staging software-pipelined into sweep 1 ---
            passes = [(tuple(range(hg * 4, hg * 4 + 4)), jc)
                      for hg in range(2) for jc in range(JCH)]
            # split the final pass into head pairs so heads 4-5 finalize
            # while 6-7 are still accumulating their last chunk
            passes[-1:] = [((4, 5), JCH - 1), ((6, 7), JCH - 1)]
            NP = len(passes)

            def route_of(p):
                heads, jc = passes[p]
                return routes.get((heads[0] // 4, jc), "D")

            g4_of = {}
            pre_done = set()

            def emit_pre(p):
                if p in pre_done or p >= NP:
                    return
                pre_done.add(p)
                heads, jc = passes[p]
                hg = heads[0] // 4
                g, o = jc // 8, jc % 8
                nh = len(heads)
                r = route_of(p)
                g4 = work.tile([128, nh * ROWS], mm_dt, tag=f"g{nh}",
                               name=f"g{nh}", bufs=8)
                g4_of[p] = g4
                if r == "D":
                    for idx, h in enumerate(heads):
                        qap = evqd_g[g][:][:, o * 8 + h:o * 8 + h + 1]
                        dap = evqd_g[g][:][:, 64 + o * 8 + h:64 + o * 8 + h + 1]
                        nc.vector.tensor_scalar(
                            g4[:][:, idx * ROWS:(idx + 1) * ROWS],
                            e8[h][:], dap, qap, OP.mult, OP.max)
                elif r == "A":
                    for idx, h in enumerate(heads):
                        qap = evqd_g[g][:][:, o * 8 + h:o * 8 + h + 1]
                        nqap = nqpat[:][:, jc * 8 + h:jc * 8 + h + 1]
                        dap = evqd_g[g][:][:, 64 + o * 8 + h:64 + o * 8 + h + 1]
                        rt = work.tile([128, ROWS], bf16, tag="art",
                                       name="art", bufs=6)
                        # r = relu(E8*expd - q); m = relu(r + q)
                        nc.scalar.activation(rt[:], e8[h][:], AF.Relu,
                                             bias=nqap, scale=dap)
                        nc.scalar.activation(
                            g4[:][:, idx * ROWS:(idx + 1) * ROWS],
                            rt[:], AF.Relu, bias=qap, scale=1.0)
                else:  # G
                    eg = egs[(hg, jc)]
                    ep = epat[:][:, jc * 8 + hg * 4:jc * 8 + hg * 4 + 4]
                    qp = qpat[:][:, jc * 8 + hg * 4:jc * 8 + hg * 4 + 4]
                    mp = work.tile([128, 4 * kpad], bf16, tag="mp",
                                   name="mp", bufs=g_ahead + 2)
                    tm = work.tile([128, 4 * kpad], bf16, tag="tm",
                                   name="tm", bufs=2)
                    nc.vector.tensor_tensor(
                        tm[:].rearrange("p (b k) -> p b k", b=4),
                        eg[:].rearrange("p (b k) -> p b k", b=4),
                        ep.unsqueeze(2).broadcast_to([128, 4, kpad]),
                        OP.mult)
                    nc.vector.tensor_tensor(
                        mp[:].rearrange("p (b k) -> p b k", b=4),
                        tm[:].rearrange("p (b k) -> p b k", b=4),
                        qp.unsqueeze(2).broadcast_to([128, 4, kpad]),
                        OP.max)
                    for half in range(2):
                        nc.gpsimd.local_scatter(
                            g4[:][:, half * 1024:(half + 1) * 1024],
                            mp[:][:, half * K2:(half + 1) * K2],
                            idxs[jc][:], channels=128, num_elems=1024,
                            num_idxs=K2)

            def emit_mask_mms(p, acc):
                heads, jc = passes[p]
                nh = len(heads)
                g4 = g4_of.pop(p)
                if route_of(p) != "G":
                    nc.vector.tensor_tensor(
                        g4[:].rearrange("p (b r) -> p b r", b=nh),
                        g4[:].rearrange("p (b r) -> p b r", b=nh),
                        adjs[jc][:].unsqueeze(1).broadcast_to([128, nh, ROWS]),
                        OP.mult)
                for idx, h in enumerate(heads):
                    nc.tensor.matmul(
                        acc[h][:],
                        wh1[jc][:][:, h * E:(h + 1) * E],
                        g4[:][:, idx * ROWS:(idx + 1) * ROWS],
                        start=(jc == 0), stop=(jc == JCH - 1))

            def emit_ahead(p):
                # route-specific lookahead: scatters/A-chains are emitted
                # further ahead of their consuming matmuls than DVE ts ops
                for lead, rts in ((g_ahead, ("G",)), (a_ahead, ("A",)),
                                  (ts_ahead, ("D",))):
                    pp = p + lead
                    if pp < NP and pp not in pre_done and route_of(pp) in rts:
                        emit_pre(pp)

            def finalize_head(h, tpp, relu_dve=False):
                tp = tpp.tile([128, 4 * E], f32, tag="tp", name="tp")
                rec = fin.tile([128, 4], f32, tag="rec", name="rec", bufs=2)
                for k in range(4):
                    ksl = slice(k * 128, (k + 1) * 128)
                    nc.tensor.transpose(
                        tp[:][:, k * E:(k + 1) * E],
                        sA[h][:][:, ksl], ident[0:E, 0:E])
                nc.vector.reciprocal(
                    rec[:],
                    tp[:].rearrange("p (k e) -> p k e", k=4)[:, :, OUT_FEAT:E])
                for k in range(4):
                    if relu_dve:
                        nc.vector.tensor_scalar(
                            outsb[k][:][:, h * OUT_FEAT:(h + 1) * OUT_FEAT],
                            tp[:][:, k * E:k * E + OUT_FEAT],
                            rec[:][:, k:k + 1], 0.0, OP.mult, OP.max)
                    else:
                        nc.scalar.activation(
                            outsb[k][:][:, h * OUT_FEAT:(h + 1) * OUT_FEAT],
                            tp[:][:, k * E:k * E + OUT_FEAT],
                            AF.Relu, scale=rec[:][:, k:k + 1])

            sA = {}
            with tc.tile_pool(name=f"stgps{rep}", bufs=1,
                              space="PSUM") as stgps:

                def stage(jc):
                    # Wh staging: matmuls + bf16 [Wh_h | 1] lhsT blocks
                    # (Wh cols on ACT -- DVE is the bottleneck engine)
                    jsl = slice(jc * 128, (jc + 1) * 128)
                    whp = stgps.tile([128, F], f32, tag="whp", name="whp",
                                     bufs=3)
                    for k in range(KCH):
                        nc.tensor.matmul(
                            whp[:], xt_sb[k][:][:, jsl], w_sb[k][:],
                            start=(k == 0), stop=(k == KCH - 1))
                    dst = wh1[jc][:].rearrange("p (h e) -> p h e", h=N_HEADS)
                    nc.scalar.copy(
                        dst[:, :, 0:OUT_FEAT],
                        whp[:].rearrange("p (h d) -> p h d", h=N_HEADS))
                    nc.scalar.copy(
                        dst[:, :, OUT_FEAT:E],
                        ones128[:].rearrange("p (e o) -> p e o", o=1))

                # prologue: e_src -> E8 broadcast, interleaved with the
                # first staged chunks so the first pass's e-vectors don't
                # queue on ACT behind all eight E8 activations. Dummy
                # transposes pre-warm the Tensor engine to full p-state
                # while the input DMAs land.
                with tc.tile_pool(name=f"props{rep}", bufs=1,
                                  space="PSUM") as props:
                    for w in range(n_warm):
                        warm = props.tile([128, 128], f32, tag="warm",
                                          name="warm", bufs=1)
                        nc.tensor.transpose(warm[:], ident[:], ident[:])

                    def bp_e8(h):
                        bp = props.tile([128, ROWS], f32, tag="bp",
                                        name="bp", bufs=4)
                        nc.tensor.matmul(bp[:],
                                         sel[:][:, h * 128:(h + 1) * 128],
                                         espall[:],
                                         start=True, stop=True)
                        nc.scalar.copy(e8[h][:], bp[:])

                    for h in range(4):
                        bp_e8(h)
                    for jc in range(lookahead):
                        stage(jc)
                    for h in range(4, 8):
                        bp_e8(h)

                with tc.tile_pool(name=f"acc{rep}_0", bufs=1,
                                  space="PSUM") as accp:
                    acc = {h: accp.tile([E, ROWS], f32, tag=f"acc{h}",
                                        name=f"acc{h}")
                           for h in range(4)}
                    for p in range(max(ts_ahead, g_ahead, a_ahead)):
                        emit_pre(p)
                    for p in range(JCH):
                        jc = passes[p][1]
                        if jc + lookahead < JCH:
                            stage(jc + lookahead)
                        emit_ahead(p)
                        emit_pre(p)  # no-op unless lookaheads missed it
                        emit_mask_mms(p, acc)
                    for h in range(4):
                        sA[h] = fin.tile([E, ROWS], f32, tag=f"sA{h % 4}",
                                         name=f"sA{h}", bufs=2)
                        nc.scalar.copy(sA[h][:], acc[h][:])

            # sweep 2 (heads 4-7); head-group-0 finalize interleaved
            fin_sched = {JCH + fin0 + fin_step * h: h for h in range(4)}
            with tc.tile_pool(name=f"acc{rep}_1", bufs=1,
                              space="PSUM") as accp, \
                 tc.tile_pool(name=f"tpp{rep}", bufs=4,
                              space="PSUM") as tpp:
                acc = {h: accp.tile([E, ROWS], f32, tag=f"acc{h}",
                                    name=f"acc{h}")
                       for h in range(4, 8)}
                dve_copy = lambda o, i: nc.vector.tensor_copy(o, i)
                sA_eng = {4: dve_copy if sa_dve else nc.scalar.copy,
                          5: nc.scalar.copy,
                          6: dve_copy if sa_dve else nc.scalar.copy,
                          7: nc.scalar.copy}

                def tail_pair(pair):
                    # PSUM->SBUF staging and the relu-scale both split
                    # across ACT/DVE so the pair's chains run in parallel
                    for h in pair:
                        sA[h] = fin.tile([E, ROWS], f32, tag=f"sA{h % 4}",
                                         name=f"sA{h}", bufs=2)
                        sA_eng[h](sA[h][:], acc[h][:])
                    for h in pair:
                        finalize_head(h, tpp, relu_dve=(h % 2 == 1))

                for p in range(JCH, NP):
                    emit_ahead(p)
                    emit_pre(p)
                    emit_mask_mms(p, acc)
                    if p in fin_sched:
                        finalize_head(fin_sched[p], tpp)
                    if p == JCH + dma0:
                        for k in range(4):
                            nc.sync.dma_start(
                                out[k * 128:(k + 1) * 128, 0:4 * OUT_FEAT],
                                outsb[k][:][:, 0:4 * OUT_FEAT])
                    if p == NP - 2:
                        tail_pair((4, 5))
                tail_pair((6, 7))
                for k in range(4):
                    nc.sync.dma_start(
                        out[k * 128:(k + 1) * 128, 4 * OUT_FEAT:F],
                        outsb[k][:][:, 4 * OUT_FEAT:F])

    nc.compile()
    return nc


_NC_CACHE = {}


def get_nc(exact=False, **kw):
    key = (exact, tuple(sorted(kw.items())))
    if key not in _NC_CACHE:
        _NC_CACHE[key] = build_nc(exact, **kw)
    return _NC_CACHE[key]


def make_in_maps(x, adj, W, a, kpad=KPAD, n_gjc=12, n_a=9):
    x = np.asarray(x, dtype=np.float32)
    adj = np.asarray(adj, dtype=np.float32)
    W = np.asarray(W, dtype=np.float32)
    a = np.asarray(a, dtype=np.float32)
    routes, g_jcs = route_plan(n_gjc, n_a)

    xT = np.ascontiguousarray(x.T)                       # [256, 4096]
    a_src = a[:, :OUT_FEAT].astype(np.float64)           # [8, 64]
    a_dst = a[:, OUT_FEAT:].astype(np.float64)
    W3 = W.astype(np.float64).reshape(IN_FEAT, N_HEADS, OUT_FEAT)
    wa_dst = np.einsum("khd,hd->kh", W3, a_dst)          # [256, 8]
    wa_src = np.einsum("khd,hd->kh", W3, a_src)

    x64 = x.astype(np.float64)
    e_dst = x64 @ wa_dst                                  # [4096, 8]
    # evqd[g*128+p, o*8+h] = exp(0.2 e_dst[j,h]); cols 64+: exp(e_dst)
    # with j = (g*8+o)*128 + p  (group/chunk/partition tile layout)
    ed = e_dst.reshape(4, 8, 128, N_HEADS).transpose(0, 2, 1, 3)
    evqd = np.stack(
        [np.exp(0.2 * ed), np.exp(ed)], axis=2).reshape(512, 128)
    sel = np.zeros((8, N_HEADS * 128), dtype=np.float32)
    for h in range(N_HEADS):
        sel[h, h * 128:(h + 1) * 128] = 1.0
    # [128, jc*8+h] patterns: exp(e_dst) / exp(0.2 e_dst), j = jc*128+p
    ed_t = e_dst.reshape(JCH, 128, N_HEADS).transpose(1, 0, 2).reshape(
        128, JCH * N_HEADS)
    epat = np.exp(ed_t).astype(ml_dtypes.bfloat16)
    qpat = np.exp(0.2 * ed_t).astype(ml_dtypes.bfloat16)
    nqpat = (-np.exp(0.2 * ed_t)).astype(np.float32)

    g_passes = [(hg, jc) for hg in range(2) for jc in range(JCH)
                if routes[(hg, jc)] == "G"]

    in_maps = []
    for c in range(N_CORES):
        rs = slice(c * ROWS, (c + 1) * ROWS)
        e_src_c = np.exp(0.8 * (x64[rs] @ wa_src))        # [512, 8] f64
        m = {
            "xT": xT,
            "esrc": np.ascontiguousarray(
                e_src_c.T.astype(np.float32)),
            "evqd": evqd.astype(np.float32),
            "W": W,
            "sel": sel,
            "adjT": np.ascontiguousarray(adj[rs, :].T).astype(ml_dtypes.bfloat16),
        }
        if n_a > 0:
            m["nqpat"] = nqpat
        if g_passes:
            m["epat"] = epat
            m["qpat"] = qpat
            e_src_bf = e_src_c.astype(ml_dtypes.bfloat16).astype(np.float32)
            adjT_c = adj[rs, :].T                          # [4096, 512]
            for jc in sorted(g_jcs):
                blk = adjT_c[jc * 128:(jc + 1) * 128, :]   # [128, 512]
                idx2 = np.full((128, 2 * kpad), -1, np.int16)
                cols_of = []
                for p in range(128):
                    cols = np.nonzero(blk[p])[0]
                    if len(cols) > kpad:
                        raise ValueError(
                            f"kpad={kpad} too small: {len(cols)} edges")
                    cols_of.append(cols)
                    idx2[p, :len(cols)] = cols
                    idx2[p, kpad:kpad + len(cols)] = cols + 512
                m[f"idx{jc}"] = idx2
                for hg in range(2):
                    if (hg, jc) not in g_passes:
                        continue
                    eg = np.zeros((128, 4 * kpad), np.float32)
                    for p in range(128):
                        cols = cols_of[p]
                        for hi in range(4):
                            eg[p, hi * kpad:hi * kpad + len(cols)] = \
                                e_src_bf[cols, hg * 4 + hi]
                    m[f"eg{hg}_{jc}"] = eg.astype(ml_dtypes.bfloat16)
        in_maps.append(m)
    return in_maps


def kernel(x, adj, W, a):
    import os
    exact = os.environ.get("GAT_EXACT", "0") == "1"
    # default: bf16 attention weights + lhsT, ~3e-3 max rel err.
    # GAT_EXACT=1: f32/f32r matmuls, ~1.5e-3, slower (all-D routes).
    nc = get_nc(exact=exact)
    in_maps = make_in_maps(x, adj, W, a, n_gjc=0 if exact else 12,
                           n_a=0 if exact else 9)
    res = run_bass_kernel_spmd(nc, in_maps, core_ids=list(range(N_CORES)))
    return np.concatenate([res.results[c]["out"] for c in range(N_CORES)],
                          axis=0)


if __name__ == "__main__":
    rng = np.random.default_rng(0)
    x = rng.standard_normal((N_NODES, IN_FEAT), dtype=np.float32)
    adj = (rng.random((N_NODES, N_NODES)) < 0.01).astype(np.float32)
    np.fill_diagonal(adj, 1.0)
    W = (rng.standard_normal((256, F), dtype=np.float32) * 0.05)
    a = rng.standard_normal((N_HEADS, 2 * OUT_FEAT), dtype=np.float32)
    out = kernel(x=x, adj=adj, W=W, a=a)
    print("out", out.shape, out.dtype, float(np.abs(out).max()))


# revision 18
# speedup vs baseline: 1.1488x; 1.1488x over previous
"""GAT layer (4096 nodes, 8 heads, 64 feat/head) on 8 Trainium2 NeuronCores.

Sharding: node dim i (rows of x/adj/output) split 8 ways; W/a replicated;
each core computes attention+aggregation for its 512-row block against all
4096 j-nodes (Wh computed redundantly per core from the full x).

Math (per core, per head h):
  s_i = e_src[i,h], d_j = e_dst[j,h]
  exp(lrelu(s+d)) = exp(0.2 s_i) * max(q_j, E8_i * expd_j)
  with q_j = exp(0.2 d_j), E8_i = exp(0.8 s_i), expd_j = exp(d_j);
  the factor exp(0.2 s_i) cancels in the softmax ratio, so
  g[j,i] = adj[i,j] * max(q_j, E8_i * expd_j) is the unnormalized attention,
  out[i,d] = relu( (sum_j g[j,i] Wh[j,d]) / (sum_j g[j,i]) ),
  both sums from ONE matmul per (j-chunk, head) with rhs=g, lhsT=[Wh_h|ones].

The [128j x 512i x 4head] g tiles are produced by THREE engine routes,
balanced so DVE / Act / Pool all finish together (the adjacency select is
the whole cost: 2 dense elementwise passes if done naively on DVE):
  D-route (DVE): dual-op tensor_scalar m = max(E8*expd, q) per head (4x
      mode, ~194ns) + one 4-head-batched mask mult g = m*adj (2x, ~1.1us).
  G-route (Pool): the ~1% adjacency density makes g tiles sparse, so
      gpsimd local_scatter builds them directly: it ZEROES the dest and
      writes only the ~5 edges/row (pad idx=-1 is ignored). The host
      pre-gathers the edge E8 values (pure indexing, no math); two tiny
      [128, 4K] DVE tensor_tensor ops (~110ns each) apply expd and the
      max-with-q on the edge values; two 2-head scatters (~1.5us each)
      materialize the masked tiles. No adj DMA for all-G chunks.
  A-route (Act): m = max(E8*expd, q) as Relu(E8*expd - q) then
      Relu(r + q) (scale/bias are per-partition APs), mask mult on DVE.
Per-route per-pass engine cost: D 1903ns DVE; G ~3.1us Pool + 220ns DVE;
A ~4.9us Act + 1.1us DVE.  The Pool engine runs ONLY local_scatter ops
mid-kernel (single ucode library -> one reload, pulled to t~1us by a dummy
scatter; identity/ones memsets run before it in the standard library).

Schedule: Wh staging (x@W + ACT copies into bf16 [Wh_h|1] lhsT blocks) is
software-pipelined into the sweep-1 passes (lookahead 4); pre-work of each
route is emitted ts_ahead/g_ahead passes early; finalize (PE transpose,
DVE reciprocal, ACT/DVE relu-scale) is interleaved into sweep 2; 4
identity-transpose warmups hold the PE p-state through the DMA window.
bf16 g/Wh keep ~3e-3 max rel error (gate 2e-2); GAT_EXACT=1 switches to
f32/f32r and routes every pass through the D path.
"""

import sys

sys.path.insert(0, "/opt/trn_rl_repo")

import ml_dtypes
import numpy as np

import concourse.bass as bass
import concourse.mybir as mybir
import concourse.tile as tile
from concourse import bacc
from concourse.bass_utils import run_bass_kernel_spmd
from concourse.masks import make_identity

N_NODES = 4096
IN_FEAT = 256
OUT_FEAT = 64
N_HEADS = 8
N_CORES = 8
ROWS = N_NODES // N_CORES          # 512 i-rows per core
F = N_HEADS * OUT_FEAT             # 512
JCH = N_NODES // 128               # 32 j-chunks
KCH = IN_FEAT // 128               # 2 k-chunks
E = OUT_FEAT + 1                   # 65: per-head lhsT cols (Wh | ones)
KPAD = 28                          # padded edge slots per (j, core-block)

f32 = mybir.dt.float32
f32r = mybir.dt.float32r
bf16 = mybir.dt.bfloat16
i16 = mybir.dt.int16
AF = mybir.ActivationFunctionType
OP = mybir.AluOpType


def route_plan(n_gjc=12, n_a=9):
    """Static per-pass route assignment. Passes are (head-group, jc) for
    hg in {0,1}, jc in 0..31, with the final pass split into head pairs.
    G is assigned by WHOLE jc (both head groups) so those chunks never
    need their adjacency tile at all."""
    if n_gjc > 0:
        g_jcs = sorted({int(round(1 + i * 29.0 / max(1, n_gjc - 1)))
                        for i in range(n_gjc)})
    else:
        g_jcs = []
    routes = {}
    for hg in range(2):
        for jc in range(JCH):
            routes[(hg, jc)] = "G" if jc in g_jcs else "D"
    # A-passes: late in each sweep, on non-G chunks, avoiding the tail
    cand = [(hg, jc) for hg in range(2) for jc in range(16, 30)
            if jc not in g_jcs]
    cand = cand[:: max(1, len(cand) // n_a)][:n_a] if n_a else []
    for key in cand:
        routes[key] = "A"
    # tail passes stay D (final chunk feeds the exit path)
    for hg in range(2):
        routes[(hg, JCH - 1)] = "D"
    return routes, g_jcs


def build_nc(exact=False, kpad=KPAD, n_gjc=12, n_a=9, lookahead=4,
             ts_ahead=2, g_ahead=4, a_ahead=3, n_warm=4, sa_dve=True,
             fin0=4, fin_step=5, dma0=26, repeat=1):
    if exact:
        n_gjc, n_a = 0, 0
    routes, g_jcs = route_plan(n_gjc, n_a)
    K2 = 2 * kpad                       # idx slots per 2-head scatter
    mm_dt = f32r if exact else bf16
    wh_dt = f32 if exact else f32r
    st_dt = f32 if exact else bf16      # x/W staging operand dtype
    nc = bacc.Bacc("TRN2", target_bir_lowering=False, debug=False,
                   num_devices=N_CORES)
    xT = nc.dram_tensor("xT", [IN_FEAT, N_NODES], st_dt, kind="ExternalInput").ap()
    esrcD = nc.dram_tensor("esrc", [8, ROWS], wh_dt, kind="ExternalInput").ap()
    evqdD = nc.dram_tensor("evqd", [N_NODES // 8, 128], f32,
                           kind="ExternalInput").ap()
    Wd = nc.dram_tensor("W", [IN_FEAT, F], st_dt, kind="ExternalInput").ap()
    selD = nc.dram_tensor("sel", [8, N_HEADS * 128], wh_dt,
                          kind="ExternalInput").ap()
    adjT = nc.dram_tensor("adjT", [N_NODES, ROWS], bf16, kind="ExternalInput").ap()
    out = nc.dram_tensor("out", [ROWS, F], f32, kind="ExternalOutput").ap()
    g_passes = [(hg, jc) for hg in range(2) for jc in range(JCH)
                if routes[(hg, jc)] == "G"]
    if g_passes:
        # edge E8 values (host-gathered), packed [pass, 128, 4*kpad] bf16
        egD = nc.dram_tensor("eg", [len(g_passes) * 128, 4 * kpad], bf16,
                             kind="ExternalInput").ap()
        idxD = nc.dram_tensor("idx", [len(g_jcs) * 128, K2], i16,
                              kind="ExternalInput").ap()
        # expd/q patterns [128, jc*8+h] bf16 (tile layout of exp(e_dst))
        epatD = nc.dram_tensor("epat", [128, JCH * 8], bf16,
                               kind="ExternalInput").ap()
        qpatD = nc.dram_tensor("qpat", [128, JCH * 8], bf16,
                               kind="ExternalInput").ap()
    if n_a > 0:
        # -q in f32 tile layout, bias AP for the A-route relu chain
        nqpatD = nc.dram_tensor("nqpat", [128, JCH * 8], f32,
                                kind="ExternalInput").ap()

    with tile.TileContext(nc) as tc:
      for rep in range(repeat):
        with tc.tile_pool(name=f"persist{rep}", bufs=1) as per, \
             tc.tile_pool(name=f"ph1sb{rep}", bufs=1) as ph1, \
             tc.tile_pool(name=f"work{rep}", bufs=1) as work, \
             tc.tile_pool(name=f"fin{rep}", bufs=1) as fin:
            # persistent SBUF (unique tags -> dedicated slots)
            wh1 = [per.tile([128, N_HEADS * E], mm_dt, tag=f"wh1_{jc}", name=f"wh1_{jc}")
                   for jc in range(JCH)]
            adjbig = per.tile([128, JCH * ROWS], bf16, tag="adjbig",
                              name="adjbig")
            adjs = {jc: adjbig[:][:, jc * ROWS:(jc + 1) * ROWS]
                    for jc in range(JCH)}
            evqdbig = per.tile([128, 4 * 128], f32, tag="evqdbig",
                               name="evqdbig")
            evqd_g = [evqdbig[:][:, g * 128:(g + 1) * 128] for g in range(4)]
            e8 = [per.tile([128, ROWS], mm_dt, tag=f"e8_{h}", name=f"e8_{h}")
                  for h in range(N_HEADS)]
            outsb = [per.tile([128, F], f32, tag=f"out_{k}", name=f"out_{k}")
                     for k in range(4)]
            espall = per.tile([8, ROWS], wh_dt, tag="espall", name="espall")
            # sel[k, h, p] = (k == h): one [8,128] slice per head is a
            # row-selector lhsT -- matmul(sel_h, espall) both selects row h
            # and broadcasts it across 128 partitions in one K=8 matmul.
            sel = per.tile([8, N_HEADS * 128], wh_dt, tag="sel", name="sel")
            ident = per.tile([128, 128], f32, tag="ident", name="ident")
            if g_passes:
                egbig = per.tile([128, len(g_passes) * 4 * kpad], bf16,
                                 tag="egbig", name="egbig")
                egs = {k: egbig[:][:, s * 4 * kpad:(s + 1) * 4 * kpad]
                       for s, k in enumerate(g_passes)}
                idxbig = per.tile([128, len(g_jcs) * K2], i16, tag="idxbig",
                                  name="idxbig")
                idxs = {jc: idxbig[:][:, s * K2:(s + 1) * K2]
                        for s, jc in enumerate(sorted(g_jcs))}
                epat = per.tile([128, JCH * 8], bf16, tag="epat", name="epat")
                qpat = per.tile([128, JCH * 8], bf16, tag="qpat", name="qpat")
                dum = per.tile([16, 2], i16, tag="dum", name="dum")
                dumo = per.tile([16, 2], bf16, tag="dumo", name="dumo")
            if n_a > 0:
                nqpat = per.tile([128, JCH * 8], f32, tag="nqpat",
                                 name="nqpat")

            xt_sb = [ph1.tile([128, N_NODES], st_dt, tag=f"xt{k}", name=f"xt{k}")
                     for k in range(KCH)]
            w_sb = [ph1.tile([128, F], st_dt, tag=f"w{k}", name=f"w{k}")
                    for k in range(KCH)]

            make_identity(nc, ident[:])
            if g_passes:
                # dummy scatter pulls the local_scatter library reload (and
                # the one-time Q7 bringup) to t~1us, before real work needs
                # the Pool engine
                nc.gpsimd.memset(dum[:], -1)
                nc.gpsimd.local_scatter(dumo[:], dum[:], dum[:], channels=16,
                                        num_elems=2, num_idxs=2)

            # --- one-time ones columns of the [Wh_h | 1] lhsT blocks:
            # DVE memsets at t=0 fill DVE's otherwise-idle DMA window and
            # delete the per-chunk ACT ones-copy from the steady state
            for jc in range(JCH):
                nc.vector.memset(
                    wh1[jc][:].rearrange("p (h e) -> p h e", h=N_HEADS)
                    [:, :, OUT_FEAT:E], 1.0)

            # --- input DMAs, all on the cheap sync queue, BATCHED (each
            # dma_start costs ~665ns of serial HWDGE descriptor-gen; the
            # grouped 3D-AP transfers cut ~85 starts to ~25) and ordered by
            # first-use time.
            def grouped(dst_tile, n_slots, src_dram, s0, s1, width):
                # SBUF DMA APs need the partition dim outermost; iterate
                # (p, slot, col) on both sides
                view = dst_tile[:].rearrange("p (s c) -> p s c",
                                             s=n_slots)[:, s0:s1]
                nc.sync.dma_start(
                    view,
                    src_dram[s0 * 128:s1 * 128, 0:width].rearrange(
                        "(s p) c -> p s c", p=128))

            nc.sync.dma_start(espall[:], esrcD)
            nc.sync.dma_start(sel[:], selD)
            grouped(evqdbig, 4, evqdD, 0, 4, 128)
            if g_passes:
                nc.sync.dma_start(epat[:], epatD)
                nc.sync.dma_start(qpat[:], qpatD)
                grouped(idxbig, len(g_jcs), idxD, 0, len(g_jcs), K2)
            if n_a > 0:
                nc.sync.dma_start(nqpat[:], nqpatD)
            grouped(adjbig, JCH, adjT, 0, 4, ROWS)          # adj chunks 0-3
            for k in range(KCH):
                sl = slice(k * 128, (k + 1) * 128)
                nc.sync.dma_start(xt_sb[k][:][:, 0:512], xT[sl, 0:512])
            grouped(w_sb[0], 1, Wd, 0, 1, F)
            grouped(w_sb[1], 1, Wd[128:256, :], 0, 1, F)
            for k in range(KCH):
                sl = slice(k * 128, (k + 1) * 128)
                nc.sync.dma_start(xt_sb[k][:][:, 512:2048], xT[sl, 512:2048])
            if g_passes:
                grouped(egbig, len(g_passes), egD, 0, len(g_passes), 4 * kpad)
            grouped(adjbig, JCH, adjT, 4, 12, ROWS)         # adj chunks 4-11
            grouped(adjbig, JCH, adjT, 12, 20, ROWS)
            for k in range(KCH):
                sl = slice(k * 128, (k + 1) * 128)
                nc.sync.dma_start(xt_sb[k][:][:, 2048:4096], xT[sl, 2048:4096])
            grouped(adjbig, JCH, adjT, 20, 28, ROWS)
            grouped(adjbig, JCH, adjT, 28, 32, ROWS)

            # --- main: two sweeps over j-chunks (head groups 0-3, 4-7),
            # Wh/e-vector staging software-pipelined into sweep 1 ---
            passes = [(tuple(range(hg * 4, hg * 4 + 4)), jc)
                      for hg in range(2) for jc in range(JCH)]
            # split the final pass into head pairs so heads 4-5 finalize
            # while 6-7 are still accumulating their last chunk
            passes[-1:] = [((4, 5), JCH - 1), ((6, 7), JCH - 1)]
            NP = len(passes)

            def route_of(p):
                heads, jc = passes[p]
                return routes.get((heads[0] // 4, jc), "D")

            g4_of = {}
            pre_done = set()

            def emit_pre(p):
                if p in pre_done or p >= NP:
                    return
                pre_done.add(p)
                heads, jc = passes[p]
                hg = heads[0] // 4
                g, o = jc // 8, jc % 8
                nh = len(heads)
                r = route_of(p)
                g4 = work.tile([128, nh * ROWS], mm_dt, tag=f"g{nh}",
                               name=f"g{nh}", bufs=8)
                g4_of[p] = g4
                if r == "D":
                    for idx, h in enumerate(heads):
                        qap = evqd_g[g][:, o * 8 + h:o * 8 + h + 1]
                        dap = evqd_g[g][:, 64 + o * 8 + h:64 + o * 8 + h + 1]
                        nc.vector.tensor_scalar(
                            g4[:][:, idx * ROWS:(idx + 1) * ROWS],
                            e8[h][:], dap, qap, OP.mult, OP.max)
                elif r == "A":
                    for idx, h in enumerate(heads):
                        qap = evqd_g[g][:, o * 8 + h:o * 8 + h + 1]
                        nqap = nqpat[:][:, jc * 8 + h:jc * 8 + h + 1]
                        dap = evqd_g[g][:, 64 + o * 8 + h:64 + o * 8 + h + 1]
                        rt = work.tile([128, ROWS], bf16, tag="art",
                                       name="art", bufs=6)
                        # r = relu(E8*expd - q); m = relu(r + q)
                        nc.scalar.activation(rt[:], e8[h][:], AF.Relu,
                                             bias=nqap, scale=dap)
                        nc.scalar.activation(
                            g4[:][:, idx * ROWS:(idx + 1) * ROWS],
                            rt[:], AF.Relu, bias=qap, scale=1.0)
                else:  # G
                    eg = egs[(hg, jc)]
                    ep = epat[:][:, jc * 8 + hg * 4:jc * 8 + hg * 4 + 4]
                    qp = qpat[:][:, jc * 8 + hg * 4:jc * 8 + hg * 4 + 4]
                    mp = work.tile([128, 4 * kpad], bf16, tag="mp",
                                   name="mp", bufs=g_ahead + 2)
                    tm = work.tile([128, 4 * kpad], bf16, tag="tm",
                                   name="tm", bufs=2)
                    nc.vector.tensor_tensor(
                        tm[:].rearrange("p (b k) -> p b k", b=4),
                        eg.rearrange("p (b k) -> p b k", b=4),
                        ep.unsqueeze(2).broadcast_to([128, 4, kpad]),
                        OP.mult)
                    nc.vector.tensor_tensor(
                        mp[:].rearrange("p (b k) -> p b k", b=4),
                        tm[:].rearrange("p (b k) -> p b k", b=4),
                        qp.unsqueeze(2).broadcast_to([128, 4, kpad]),
                        OP.max)
                    for half in range(2):
                        nc.gpsimd.local_scatter(
                            g4[:][:, half * 1024:(half + 1) * 1024],
                            mp[:][:, half * K2:(half + 1) * K2],
                            idxs[jc], channels=128, num_elems=1024,
                            num_idxs=K2)

            def emit_mask_mms(p, acc):
                heads, jc = passes[p]
                nh = len(heads)
                g4 = g4_of.pop(p)
                if route_of(p) != "G":
                    nc.vector.tensor_tensor(
                        g4[:].rearrange("p (b r) -> p b r", b=nh),
                        g4[:].rearrange("p (b r) -> p b r", b=nh),
                        adjs[jc].unsqueeze(1).broadcast_to([128, nh, ROWS]),
                        OP.mult)
                for idx, h in enumerate(heads):
                    nc.tensor.matmul(
                        acc[h][:],
                        wh1[jc][:][:, h * E:(h + 1) * E],
                        g4[:][:, idx * ROWS:(idx + 1) * ROWS],
                        start=(jc == 0), stop=(jc == JCH - 1))

            def emit_ahead(p):
                # route-specific lookahead: scatters/A-chains are emitted
                # further ahead of their consuming matmuls than DVE ts ops
                for lead, rts in ((g_ahead, ("G",)), (a_ahead, ("A",)),
                                  (ts_ahead, ("D",))):
                    pp = p + lead
                    if pp < NP and pp not in pre_done and route_of(pp) in rts:
                        emit_pre(pp)

            def finalize_head(h, tpp, relu_dve=False):
                tp = tpp.tile([128, 4 * E], f32, tag="tp", name="tp")
                rec = fin.tile([128, 4], f32, tag="rec", name="rec", bufs=2)
                for k in range(4):
                    ksl = slice(k * 128, (k + 1) * 128)
                    nc.tensor.transpose(
                        tp[:][:, k * E:(k + 1) * E],
                        sA[h][:][:, ksl], ident[0:E, 0:E])
                nc.vector.reciprocal(
                    rec[:],
                    tp[:].rearrange("p (k e) -> p k e", k=4)[:, :, OUT_FEAT:E])
                for k in range(4):
                    if relu_dve:
                        nc.vector.tensor_scalar(
                            outsb[k][:][:, h * OUT_FEAT:(h + 1) * OUT_FEAT],
                            tp[:][:, k * E:k * E + OUT_FEAT],
                            rec[:][:, k:k + 1], 0.0, OP.mult, OP.max)
                    else:
                        nc.scalar.activation(
                            outsb[k][:][:, h * OUT_FEAT:(h + 1) * OUT_FEAT],
                            tp[:][:, k * E:k * E + OUT_FEAT],
                            AF.Relu, scale=rec[:][:, k:k + 1])

            sA = {}
            with tc.tile_pool(name=f"stgps{rep}", bufs=1,
                              space="PSUM") as stgps:

                def stage(jc):
                    # Wh staging: matmuls + bf16 [Wh_h | 1] lhsT blocks
                    # (Wh cols on ACT -- DVE is the bottleneck engine)
                    jsl = slice(jc * 128, (jc + 1) * 128)
                    whp = stgps.tile([128, F], f32, tag="whp", name="whp",
                                     bufs=3)
                    for k in range(KCH):
                        nc.tensor.matmul(
                            whp[:], xt_sb[k][:][:, jsl], w_sb[k][:],
                            start=(k == 0), stop=(k == KCH - 1))
                    dst = wh1[jc][:].rearrange("p (h e) -> p h e", h=N_HEADS)
                    nc.scalar.copy(
                        dst[:, :, 0:OUT_FEAT],
                        whp[:].rearrange("p (h d) -> p h d", h=N_HEADS))

                # prologue: e_src -> E8 broadcast, interleaved with the
                # first staged chunks so the first pass's e-vectors don't
                # queue on ACT behind all eight E8 activations. Dummy
                # transposes pre-warm the Tensor engine to full p-state
                # while the input DMAs land.
                with tc.tile_pool(name=f"props{rep}", bufs=1,
                                  space="PSUM") as props:
                    for w in range(n_warm):
                        warm = props.tile([128, 128], f32, tag="warm",
                                          name="warm", bufs=1)
                        nc.tensor.transpose(warm[:], ident[:], ident[:])

                    def bp_e8(h):
                        bp = props.tile([128, ROWS], f32, tag="bp",
                                        name="bp", bufs=4)
                        nc.tensor.matmul(bp[:],
                                         sel[:][:, h * 128:(h + 1) * 128],
                                         espall[:],
                                         start=True, stop=True)
                        nc.vector.tensor_copy(e8[h][:], bp[:])

                    for h in range(4):
                        bp_e8(h)
                    for jc in range(lookahead):
                        stage(jc)
                    for h in range(4, 8):
                        bp_e8(h)

                with tc.tile_pool(name=f"acc{rep}_0", bufs=1,
                                  space="PSUM") as accp:
                    acc = {h: accp.tile([E, ROWS], f32, tag=f"acc{h}",
                                        name=f"acc{h}")
                           for h in range(4)}
                    for p in range(max(ts_ahead, g_ahead, a_ahead)):
                        emit_pre(p)
                    for p in range(JCH):
                        jc = passes[p][1]
                        if jc + lookahead < JCH:
                            stage(jc + lookahead)
                        emit_ahead(p)
                        emit_pre(p)  # no-op unless lookaheads missed it
                        emit_mask_mms(p, acc)
                    for h in range(4):
                        sA[h] = fin.tile([E, ROWS], f32, tag=f"sA{h % 4}",
                                         name=f"sA{h}", bufs=2)
                        nc.scalar.copy(sA[h][:], acc[h][:])

            # sweep 2 (heads 4-7); head-group-0 finalize interleaved
            fin_sched = {JCH + fin0 + fin_step * h: h for h in range(4)}
            with tc.tile_pool(name=f"acc{rep}_1", bufs=1,
                              space="PSUM") as accp, \
                 tc.tile_pool(name=f"tpp{rep}", bufs=4,
                              space="PSUM") as tpp:
                acc = {h: accp.tile([E, ROWS], f32, tag=f"acc{h}",
                                    name=f"acc{h}")
                       for h in range(4, 8)}
                dve_copy = lambda o, i: nc.vector.tensor_copy(o, i)
                sA_eng = {4: dve_copy if sa_dve else nc.scalar.copy,
                          5: nc.scalar.copy,
                          6: dve_copy if sa_dve else nc.scalar.copy,
                          7: nc.scalar.copy}

                def tail_pair(pair):
                    # PSUM->SBUF staging and the relu-scale both split
                    # across ACT/DVE so the pair's chains run in parallel
                    for h in pair:
                        sA[h] = fin.tile([E, ROWS], f32, tag=f"sA{h % 4}",
                                         name=f"sA{h}", bufs=2)
                        sA_eng[h](sA[h][:], acc[h][:])
                    for h in pair:
                        finalize_head(h, tpp, relu_dve=(h % 2 == 1))

                for p in range(JCH, NP):
                    emit_ahead(p)
                    emit_pre(p)
                    emit_mask_mms(p, acc)
                    if p in fin_sched:
                        finalize_head(fin_sched[p], tpp)
                    if p == JCH + dma0:
                        for k in range(4):
                            nc.sync.dma_start(
                                out[k * 128:(k + 1) * 128, 0:4 * OUT_FEAT],
                                outsb[k][:][:, 0:4 * OUT_FEAT])
                    if p == NP - 2:
                        tail_pair((4, 5))
                tail_pair((6, 7))
                for k in range(4):
                    nc.sync.dma_start(
                        out[k * 128:(k + 1) * 128, 4 * OUT_FEAT:F],
                        outsb[k][:][:, 4 * OUT_FEAT:F])

    nc.compile()
    return nc


_NC_CACHE = {}


def get_nc(exact=False, **kw):
    key = (exact, tuple(sorted(kw.items())))
    if key not in _NC_CACHE:
        _NC_CACHE[key] = build_nc(exact, **kw)
    return _NC_CACHE[key]


def make_in_maps(x, adj, W, a, kpad=KPAD, n_gjc=12, n_a=9, exact=False):
    x = np.asarray(x, dtype=np.float32)
    adj = np.asarray(adj, dtype=np.float32)
    W = np.asarray(W, dtype=np.float32)
    a = np.asarray(a, dtype=np.float32)
    routes, g_jcs = route_plan(n_gjc, n_a)

    xT = np.ascontiguousarray(x.T)                       # [256, 4096]
    a_src = a[:, :OUT_FEAT].astype(np.float64)           # [8, 64]
    a_dst = a[:, OUT_FEAT:].astype(np.float64)
    W3 = W.astype(np.float64).reshape(IN_FEAT, N_HEADS, OUT_FEAT)
    wa_dst = np.einsum("khd,hd->kh", W3, a_dst)          # [256, 8]
    wa_src = np.einsum("khd,hd->kh", W3, a_src)

    x64 = x.astype(np.float64)
    e_dst = x64 @ wa_dst                                  # [4096, 8]
    # evqd[g*128+p, o*8+h] = exp(0.2 e_dst[j,h]); cols 64+: exp(e_dst)
    # with j = (g*8+o)*128 + p  (group/chunk/partition tile layout)
    ed = e_dst.reshape(4, 8, 128, N_HEADS).transpose(0, 2, 1, 3)
    evqd = np.stack(
        [np.exp(0.2 * ed), np.exp(ed)], axis=2).reshape(512, 128)
    sel = np.zeros((8, N_HEADS * 128), dtype=np.float32)
    for h in range(N_HEADS):
        sel[h, h * 128:(h + 1) * 128] = 1.0
    # [128, jc*8+h] patterns: exp(e_dst) / exp(0.2 e_dst), j = jc*128+p
    ed_t = e_dst.reshape(JCH, 128, N_HEADS).transpose(1, 0, 2).reshape(
        128, JCH * N_HEADS)
    epat = np.exp(ed_t).astype(ml_dtypes.bfloat16)
    qpat = np.exp(0.2 * ed_t).astype(ml_dtypes.bfloat16)
    nqpat = (-np.exp(0.2 * ed_t)).astype(np.float32)

    g_passes = [(hg, jc) for hg in range(2) for jc in range(JCH)
                if routes[(hg, jc)] == "G"]
    sdt = np.float32 if exact else ml_dtypes.bfloat16

    in_maps = []
    for c in range(N_CORES):
        rs = slice(c * ROWS, (c + 1) * ROWS)
        e_src_c = np.exp(0.8 * (x64[rs] @ wa_src))        # [512, 8] f64
        m = {
            "xT": xT.astype(sdt),
            "esrc": np.ascontiguousarray(
                e_src_c.T.astype(np.float32)),
            "evqd": evqd.astype(np.float32),
            "W": W.astype(sdt),
            "sel": sel,
            "adjT": np.ascontiguousarray(adj[rs, :].T).astype(ml_dtypes.bfloat16),
        }
        if n_a > 0:
            m["nqpat"] = nqpat
        if g_passes:
            m["epat"] = epat
            m["qpat"] = qpat
            e_src_bf = e_src_c.astype(ml_dtypes.bfloat16).astype(np.float32)
            adjT_c = adj[rs, :].T                          # [4096, 512]
            cols_of = {}
            idx_all = np.full((len(g_jcs) * 128, 2 * kpad), -1, np.int16)
            for s, jc in enumerate(sorted(g_jcs)):
                blk = adjT_c[jc * 128:(jc + 1) * 128, :]   # [128, 512]
                for p in range(128):
                    cols = np.nonzero(blk[p])[0]
                    if len(cols) > kpad:
                        raise ValueError(
                            f"kpad={kpad} too small: {len(cols)} edges")
                    cols_of[(jc, p)] = cols
                    idx_all[s * 128 + p, :len(cols)] = cols
                    idx_all[s * 128 + p, kpad:kpad + len(cols)] = cols + 512
            m["idx"] = idx_all
            eg_all = np.zeros((len(g_passes) * 128, 4 * kpad), np.float32)
            for s, (hg, jc) in enumerate(g_passes):
                for p in range(128):
                    cols = cols_of[(jc, p)]
                    for hi in range(4):
                        eg_all[s * 128 + p, hi * kpad:hi * kpad + len(cols)] \
                            = e_src_bf[cols, hg * 4 + hi]
            m["eg"] = eg_all.astype(ml_dtypes.bfloat16)
        in_maps.append(m)
    return in_maps


def kernel(x, adj, W, a):
    import os
    exact = os.environ.get("GAT_EXACT", "0") == "1"
    # default: bf16 attention weights + lhsT, ~3e-3 max rel err.
    # GAT_EXACT=1: f32/f32r matmuls, ~1.5e-3, slower (all-D routes).
    nc = get_nc(exact=exact)
    in_maps = make_in_maps(x, adj, W, a, n_gjc=0 if exact else 12,
                           n_a=0 if exact else 9, exact=exact)
    res = run_bass_kernel_spmd(nc, in_maps, core_ids=list(range(N_CORES)))
    return np.concatenate([res.results[c]["out"] for c in range(N_CORES)],
                          axis=0)


if __name__ == "__main__":
    rng = np.random.default_rng(0)
    x = rng.standard_normal((N_NODES, IN_FEAT), dtype=np.float32)
    adj = (rng.random((N_NODES, N_NODES)) < 0.01).astype(np.float32)
    np.fill_diagonal(adj, 1.0)
    W = (rng.standard_normal((256, F), dtype=np.float32) * 0.05)
    a = rng.standard_normal((N_HEADS, 2 * OUT_FEAT), dtype=np.float32)
    out = kernel(x=x, adj=adj, W=W, a=a)
    print("out", out.shape, out.dtype, float(np.abs(out).max()))
